# revision 34
# baseline (speedup 1.0000x reference)
"""GAT (3 convs) + Set2Set + MLP on 8 Trainium2 NeuronCores.

v3 design:
- Phase 1: each core computes xl = h @ [W0|W1|w_as|w_ad] for its OWN
  6250-node shard, writing bf16 768B rows [xl0|1|xl1|1|a_src0|a_src1|pad]
  plus a local 256B-row a_dst table; one AllGather distributes the rows.
- Edge phase per 128-dst window: SWDGE dma_gather of the 768B rows keyed
  by src (int16 idx over two 4-core halves) + one SWDGE gather of a_dst
  256B rows keyed by local dst; batched leaky/exp per window; per
  128-edge segment ONE bf16 one-hot (DVE) + two scaled copies (ACT/DVE)
  + ONE 258-col bf16 matmul accumulating both heads' messages AND
  softmax denominators (via the in-row ones scaled by exp).
- All per-edge index tables are conv-invariant and live in SBUF for the
  whole kernel (loaded once). h stays transposed in SBUF between convs.
- Set2Set + MLP per core on its 16-graph slice (as before).
"""
import os
import sys

import numpy as np

sys.path.insert(0, "/opt/trn_rl_repo")

N, E, F_RAW, D, H, B = 50000, 800000, 9, 128, 2, 128
NUM_CONVS = int(os.environ.get("K_CONVS", "3"))
AGGR_STEPS = int(os.environ.get("K_STEPS", "3"))
NEG_SLOPE = 0.2
NCORES = 8
SHARD = N // NCORES            # 6250
HALF = N // 2                  # 25000
NW = (SHARD + 127) // 128      # 49 windows per core
LASTW = SHARD - (NW - 1) * 128 # 106
XROW = 384                     # bf16 cols per xl row -> 768B
OWNROWS = NW * 128             # 6272

_cached = {}


# ---------------------------------------------------------------- patches
def _install_patches():
    import concourse.tile as tile_mod
    from concourse.vector_clock import ScopedClock, VectorClock

    if not getattr(tile_mod.TileContext, "_drain_patched", False):
        def patched(self, tick_clock, wait_clock):
            gc = tick_clock.global_clock
            vals = [gc[p] for p in range(27)]
            for p in [p for p in range(27) if vals[p] > 0]:
                sub = [vals[q] if q == p else 0 for q in range(27)]
                nop = self.nc.sync.nop(nofuse=True, hint="drain_wait_split")
                wait_clock.add_sem_waits(
                    nop.ins, ScopedClock({None: VectorClock(sub)}))
            self.nc.sync.drain()
            self.nc.all_engine_barrier()
            popped = self.nc._tile_sem_poison_stack.pop()
            assert popped is self._sem_poison
            self.nc.clear_and_free_semaphores(
                list(self.sems.allocated().values()))
            self.nc.all_engine_barrier()

        tile_mod.TileContext._drain_and_barrier = patched
        tile_mod.TileContext._drain_patched = True


def _split_waits(nc, max_waits=1):
    """walrus allows at most one sync-wait command per instruction;
    spread extras across injected same-engine NoOps."""
    from concourse import mybir
    n = 0
    for f in nc.m.functions:
        for bb in f.blocks:
            changed, new = False, []
            for ins in bb.instructions:
                si = ins.sync_info
                if si is not None and len(si.on_wait) > max_waits:
                    waits = list(si.on_wait)
                    for i, w in enumerate(waits[max_waits:]):
                        nop = mybir.InstNoOp(
                            name=f"{ins.name}-ws{i}", ins=[], outs=[])
                        nop.engine = ins.engine
                        nop.sync_info = mybir.SyncInfo(
                            on_wait=[w], on_update=[])
                        new.append(nop)
                    ins.sync_info = mybir.SyncInfo(
                        on_wait=waits[:max_waits],
                        on_update=list(si.on_update))
                    changed = True
                    n += 1
                new.append(ins)
            if changed:
                bb.instructions = new
    return n


# ---------------------------------------------------------------- host prep
def _wrap16(flat):
    """dma_gather index layout: idx k at [k%16, k//16], replicated to 128."""
    k = flat.shape[0]
    w = flat.reshape(k // 16, 16).T.astype(np.int16)
    return np.tile(w, (8, 1))


def _host_prep(x, edge_index, batch_index):
    import ml_dtypes
    cfg = {}
    src = np.concatenate([edge_index[0], np.arange(N, dtype=np.int64)])
    dst = np.concatenate([edge_index[1], np.arange(N, dtype=np.int64)])
    order = np.argsort(dst, kind="stable")
    src, dst = src[order].astype(np.int64), dst[order].astype(np.int64)

    core_of = dst // SHARD
    win_of = (dst % SHARD) // 128
    half_of = (src >= HALF).astype(np.int64)
    key = (core_of * NW + win_of) * 2 + half_of
    korder = np.argsort(key, kind="stable")
    src_s, dst_s = src[korder], dst[korder]
    counts = np.bincount(key[korder], minlength=NCORES * NW * 2).reshape(
        NCORES, NW, 2)
    SA = int(np.ceil(counts[:, :, 0].max() / 128))
    SB = int(np.ceil(counts[:, :, 1].max() / 128))
    SW = SA + SB
    cfg["SA"], cfg["SB"], cfg["SW"] = SA, SB, SW

    starts = np.zeros(NCORES * NW * 2 + 1, np.int64)
    np.cumsum(counts.reshape(-1), out=starts[1:])

    # graph boundaries for set2set
    goff = np.searchsorted(batch_index, np.arange(B + 1))
    rows_per_core = np.array(
        [goff[16 * (c + 1)] - goff[16 * c] for c in range(NCORES)])
    T = int(np.ceil(rows_per_core.max() / 128))
    cfg["T"] = T

    # gather row index within 4-core half table
    grow = (src_s // SHARD) % 4 * OWNROWS + src_s % SHARD

    per_core = []
    for c in range(NCORES):
        eidx = np.zeros((NW, 128, SW * 8), np.int16)
        didx = np.zeros((NW, 128, SW * 8), np.int16)
        dlb = np.full((NW, 128, SW), -1.0, np.float32)
        for w in range(NW):
            dflat = np.zeros(SW * 128, np.int64)
            for hf, (S_h, sbase) in enumerate(((SA, 0), (SB, SA))):
                k = (c * NW + w) * 2 + hf
                lo, hi = starts[k], starts[k + 1]
                cnt = hi - lo
                flat = np.zeros(S_h * 128, np.int64)
                flat[:cnt] = grow[lo:hi]
                eidx[w, :, sbase * 8:(sbase + S_h) * 8] = _wrap16(flat)
                dl = np.full(S_h * 128, -1.0, np.float32)
                dl[:cnt] = (dst_s[lo:hi] % SHARD - 128 * w).astype(np.float32)
                dfl = np.zeros(S_h * 128, np.int64)
                dfl[:cnt] = dst_s[lo:hi] % SHARD
                sl = slice(sbase, sbase + S_h)
                dlb[w, :, sl] = dl.reshape(S_h, 128).T
                dflat[sbase * 128:(sbase + S_h) * 128] = dfl
            didx[w] = _wrap16(dflat)

        eidx = np.ascontiguousarray(eidx.transpose(1, 0, 2)).reshape(128, -1)
        didx = np.ascontiguousarray(didx.transpose(1, 0, 2)).reshape(128, -1)
        dlb = np.ascontiguousarray(dlb.transpose(1, 0, 2)).reshape(128, -1)

        # set2set slice
        r0, r1 = goff[16 * c], goff[16 * (c + 1)]
        xidx = np.zeros((T, 128, 1), np.int32)
        bloc = np.full((T, 128, 1), -1.0, np.float32)
        rows = np.arange(T * 128)
        glob = np.minimum(r0 + rows, N - 1)
        xidx[:, :, 0] = glob.reshape(T, 128)
        valid = (r0 + rows) < r1
        bl = np.full(T * 128, -1.0, np.float32)
        bl[valid] = (batch_index[glob[valid]] - 16 * c).astype(np.float32)
        bloc[:, :, 0] = bl.reshape(T, 128)
        brep = np.tile(bl.reshape(T, 1, 128), (1, 16, 1)).astype(np.float32)

        per_core.append(dict(
            eidx=eidx, didx=didx, dlb=dlb,
            s2s_xidx=xidx, s2s_bloc=bloc, s2s_brep=brep,
        ))
    return cfg, per_core


def _prep_weights(x, gat_W, gat_att_src, gat_att_dst):
    import ml_dtypes
    W = np.asarray(gat_W, np.float32)              # [128, 256]
    asrc_v = np.asarray(gat_att_src, np.float32)   # [2, 128]
    adst_v = np.asarray(gat_att_dst, np.float32)
    w_as = np.stack([W[:, h * D:(h + 1) * D] @ asrc_v[h] for h in range(H)],
                    axis=1)                        # [128, 2]
    w_ad = np.stack([W[:, h * D:(h + 1) * D] @ adst_v[h] for h in range(H)],
                    axis=1)
    W_eff = np.concatenate([W, w_as, w_ad], axis=1)  # [128, 260]
    W_eff = W_eff.astype(ml_dtypes.bfloat16)

    xp = np.zeros((N, D), np.float32)
    xp[:, :F_RAW] = x
    h0T = np.zeros((NCORES, 128, OWNROWS), ml_dtypes.bfloat16)
    for c in range(NCORES):
        h0T[c, :, :SHARD] = xp[SHARD * c:SHARD * (c + 1)].T
    return W_eff, h0T


# ---------------------------------------------------------------- device build
def _build(cfg):
    import concourse.bacc as bacc
    import concourse.bass as bass
    import concourse.tile as tile
    from concourse import mybir
    from concourse.masks import make_identity

    _install_patches()
    f32 = mybir.dt.float32
    bf16 = mybir.dt.bfloat16
    AF = mybir.ActivationFunctionType
    OP = mybir.AluOpType
    SA, SB, SW, T = cfg["SA"], cfg["SB"], cfg["SW"], cfg["T"]

    DEBUG = bool(int(os.environ.get("K_DEBUG", "0")))
    nc = bacc.Bacc("TRN2", num_swdge_queues=4)
    P_ = nc.declare_dram_parameter
    h0T = P_("h0T", [128, OWNROWS], bf16, isOutput=False)
    W_eff = P_("W_eff", [128, 260], bf16, isOutput=False)
    bias_rep = P_("bias_rep", [128, 128], f32, isOutput=False)
    eidx = P_("eidx", [128, NW * SW * 8], mybir.dt.int16, isOutput=False)
    dlb = P_("dlb", [128, NW * SW], f32, isOutput=False)
    s2s_xidx = P_("s2s_xidx", [T, 128, 1], mybir.dt.int32, isOutput=False)
    s2s_bloc = P_("s2s_bloc", [T, 128, 1], f32, isOutput=False)
    s2s_brep = P_("s2s_brep", [T, 16, 128], f32, isOutput=False)
    WihT_a = P_("WihT_a", [128, 512], f32, isOutput=False)
    WihT_b = P_("WihT_b", [128, 512], f32, isOutput=False)
    WhhT = P_("WhhT", [128, 512], f32, isOutput=False)
    bg_rep = P_("bg_rep", [16, 512], f32, isOutput=False)
    W1a = P_("W1a", [128, 128], f32, isOutput=False)
    W1b = P_("W1b", [128, 128], f32, isOutput=False)
    W2 = P_("W2", [128, 128], f32, isOutput=False)
    b1_rep = P_("b1_rep", [16, 128], f32, isOutput=False)
    b2_rep = P_("b2_rep", [16, 128], f32, isOutput=False)
    out = P_("out", [16, 128], f32, isOutput=True)
    if DEBUG:
        dbg_ps = P_("dbg_ps", [128, 260], f32, isOutput=True)
        dbg_xl = P_("dbg_xl", [128, XROW], bf16, isOutput=True)
        dbg_g = P_("dbg_g", [128, XROW], bf16, isOutput=True)
        dbg_adt = P_("dbg_adt", [128, 2 * SW], bf16, isOutput=True)
        dbg_ex = P_("dbg_ex", [128, 2 * SW], f32, isOutput=True)
        dbg_oh = P_("dbg_oh", [128, 128], bf16, isOutput=True)
        dbg_gsc = P_("dbg_gsc", [128, 258], bf16, isOutput=True)
        dbg_pagg = P_("dbg_pagg", [128, 258], f32, isOutput=True)
        dbg_hn = P_("dbg_hn", [128, 128], f32, isOutput=True)
        dbg_hn1 = P_("dbg_hn1", [128, 128], f32, isOutput=True)
        dbg_hn2 = P_("dbg_hn2", [128, 128], f32, isOutput=True)
        dbg_h3 = P_("dbg_h3", [128, 128], f32, isOutput=True)

    xl_own = nc.dram_tensor("xl_own", [OWNROWS, XROW], bf16)
    xl_all = nc.dram_tensor("xl_all", [NCORES * OWNROWS, XROW], bf16,
                            addr_space="Shared")
    h_sh = nc.dram_tensor("h_sh", [SHARD, 128], f32)
    h3_full = nc.dram_tensor("h3_full", [N, 128], f32, addr_space="Shared")

    qrot = [0]

    with tile.TileContext(nc) as tc:
        with tc.tile_pool(name="consts", bufs=1) as cp:
            ident = cp.tile([128, 128], f32)
            make_identity(nc, ident[:])
            ident_b = cp.tile([128, 128], bf16)
            make_identity(nc, ident_b[:])
            adst_sb = cp.tile([128, NW, 2], bf16)
            iota_row = cp.tile([128, 128], f32)    # [p, j] = j
            nc.gpsimd.iota(iota_row[:], pattern=[[1, 128]], base=0,
                           channel_multiplier=0,
                           allow_small_or_imprecise_dtypes=True)
            iota16_row = cp.tile([128, 16], f32)
            nc.gpsimd.iota(iota16_row[:], pattern=[[1, 16]], base=0,
                           channel_multiplier=0,
                           allow_small_or_imprecise_dtypes=True)
            iota16_col = cp.tile([16, 1], f32)
            nc.gpsimd.iota(iota16_col[:], pattern=[[0, 1]], base=0,
                           channel_multiplier=1,
                           allow_small_or_imprecise_dtypes=True)
            negones_row = cp.tile([1, 128], f32)
            nc.vector.memset(negones_row[:], -1.0)
            weff_sb = cp.tile([128, 260], bf16)
            nc.sync.dma_start(out=weff_sb[:], in_=W_eff[:])
            bias_sb = cp.tile([128, 128], f32)
            nc.sync.dma_start(out=bias_sb[:], in_=bias_rep[:])
            hT_sb = cp.tile([128, OWNROWS], bf16)
            nc.sync.dma_start(out=hT_sb[:], in_=h0T[:])
            # conv-invariant edge tables, resident for the whole kernel
            eidx_sb = cp.tile([128, NW * SW * 8], mybir.dt.int16)
            nc.sync.dma_start(out=eidx_sb[:], in_=eidx[:])
            dlb_sb = cp.tile([128, NW * SW], f32)
            nc.sync.dma_start(out=dlb_sb[:], in_=dlb[:])
            wia = cp.tile([128, 512], f32)
            nc.sync.dma_start(out=wia[:], in_=WihT_a[:])
            wib = cp.tile([128, 512], f32)
            nc.sync.dma_start(out=wib[:], in_=WihT_b[:])
            whh = cp.tile([128, 512], f32)
            nc.sync.dma_start(out=whh[:], in_=WhhT[:])
            bg_sb = cp.tile([16, 512], f32)
            nc.sync.dma_start(out=bg_sb[:], in_=bg_rep[:])
            w1a_sb = cp.tile([128, 128], f32)
            nc.sync.dma_start(out=w1a_sb[:], in_=W1a[:])
            w1b_sb = cp.tile([128, 128], f32)
            nc.sync.dma_start(out=w1b_sb[:], in_=W1b[:])
            w2_sb = cp.tile([128, 128], f32)
            nc.sync.dma_start(out=w2_sb[:], in_=W2[:])
            b1_sb = cp.tile([16, 128], f32)
            nc.sync.dma_start(out=b1_sb[:], in_=b1_rep[:])
            b2_sb = cp.tile([16, 128], f32)
            nc.sync.dma_start(out=b2_sb[:], in_=b2_rep[:])

            for conv in range(NUM_CONVS):
                # ---- phase 1: own-shard xl_ext = h @ W_eff ----
                with tc.tile_pool(name="p1s", bufs=3) as p1s, \
                     tc.tile_pool(name="p1p", bufs=3, space="PSUM") as p1p:
                    for t in range(NW):
                        nwn = 128 if t < NW - 1 else LASTW
                        ps = p1p.tile([128, 260], f32, tag="p1")
                        nc.tensor.matmul(ps[0:nwn, :],
                                         lhsT=hT_sb[:, 128 * t:128 * t + nwn],
                                         rhs=weff_sb[:], start=True, stop=True)
                        xo = p1s.tile([128, 260], bf16, tag="xo")
                        nc.scalar.copy(xo[0:nwn, 0:128], ps[0:nwn, 0:128])
                        nc.scalar.copy(xo[0:nwn, 129:257], ps[0:nwn, 128:256])
                        nc.vector.tensor_copy(xo[0:nwn, 258:260],
                                              ps[0:nwn, 256:258])
                        nc.vector.memset(
                            xo[0:nwn, 0:258].rearrange(
                                "p (a b) -> p a b", b=129)[:, :, 128:129], 1.0)
                        nc.vector.tensor_copy(adst_sb[0:nwn, t, :],
                                              ps[0:nwn, 258:260])
                        nc.sync.dma_start(
                            out=xl_own[128 * t:128 * t + nwn, 0:260],
                            in_=xo[0:nwn, :])
                        if DEBUG and conv == 0 and t == 0:
                            psc = p1s.tile([128, 260], f32, tag="psc")
                            nc.vector.tensor_copy(psc[:], ps[:])
                            nc.sync.dma_start(out=dbg_ps[:], in_=psc[:])

                if DEBUG and conv == 0:
                    nc.sync.dma_start(out=dbg_xl[:], in_=xl_own[0:128, :])

                nc.gpsimd.collective_compute(
                    "AllGather", mybir.AluOpType.bypass,
                    ins=[xl_own[:]], outs=[xl_all[:]],
                    replica_groups=[list(range(NCORES))])

                # ---- edge phase: one window of 128 dst nodes at a time ----
                with tc.tile_pool(name="eg", bufs=2) as eg, \
                     tc.tile_pool(name="em", bufs=2) as em, \
                     tc.tile_pool(name="eo", bufs=3) as eo, \
                     tc.tile_pool(name="egs", bufs=3) as egs, \
                     tc.tile_pool(name="agg", bufs=2, space="PSUM") as aggp, \
                     tc.tile_pool(name="epe", bufs=2, space="PSUM") as epep, \
                     tc.tile_pool(name="etp", bufs=2, space="PSUM") as etp:
                    for w in range(NW):
                        nwn = 128 if w < NW - 1 else LASTW
                        it = eidx_sb[:, w * SW * 8:(w + 1) * SW * 8]
                        dlf = dlb_sb[:, w * SW:(w + 1) * SW]

                        g = eg.tile([128, SW, XROW], bf16, tag="g")
                        for (sbase, S_h, rbase) in (
                                (0, SA, 0), (SA, SB, 4 * OWNROWS)):
                            h1 = (S_h + 1) // 2
                            for (s0, ns) in ((0, h1), (h1, S_h - h1)):
                                if ns == 0:
                                    continue
                                q = qrot[0] % 4
                                qrot[0] += 1
                                nc.gpsimd.dma_gather(
                                    out_ap=g[:, sbase + s0:sbase + s0 + ns, :],
                                    in_ap=xl_all[rbase:rbase + 4 * OWNROWS, :],
                                    idxs_ap=it[:, (sbase + s0) * 8:
                                               (sbase + s0 + ns) * 8],
                                    num_idxs=ns * 128,
                                    num_idxs_reg=ns * 128,
                                    elem_size=XROW, queue_num=q)
                        # per-edge a_dst via PE: transpose each one-hot and
                        # matmul against this window's a_dst pairs (SBUF)
                        oh_all = em.tile([128, SW, 128], bf16, tag="oha")
                        pe_all = epep.tile([128, 2 * SW], f32, tag="pea")
                        for s in range(SW):
                            nc.vector.tensor_scalar(
                                out=oh_all[:, s, :], in0=iota_row[:],
                                scalar1=dlf[:, s:s + 1], scalar2=None,
                                op0=OP.is_equal)
                            ohp = etp.tile([128, 128], bf16, tag="ohp")
                            nc.tensor.transpose(ohp[:], oh_all[:, s, :],
                                                ident_b[:])
                            ohT = eo.tile([128, 128], bf16, tag="ohT")
                            nc.scalar.copy(ohT[:], ohp[:])
                            nc.tensor.matmul(pe_all[:, 2 * s:2 * s + 2],
                                             lhsT=ohT[:],
                                             rhs=adst_sb[:, w, :],
                                             start=True, stop=True)
                        pe_sb = em.tile([128, SW, 2], bf16, tag="pesb")
                        nc.scalar.copy(
                            pe_sb.rearrange("p s c -> p (s c)"), pe_all[:])

                        lg = em.tile([128, SW, 2], f32, tag="lg")
                        nc.vector.tensor_tensor(out=lg[:],
                                                in0=g[:, :, 258:260],
                                                in1=pe_sb[:], op=OP.add)
                        lr = em.tile([128, SW, 2], f32, tag="lr")
                        nc.vector.scalar_tensor_tensor(
                            out=lr[:], in0=lg[:], scalar=NEG_SLOPE,
                            in1=lg[:], op0=OP.mult, op1=OP.max)
                        ex = em.tile([128, SW, 2], f32, tag="ex")
                        nc.scalar.activation(
                            ex.rearrange("p s c -> p (s c)"),
                            lr.rearrange("p s c -> p (s c)"), AF.Exp)
                        if DEBUG and conv == 0 and w == 0:
                            nc.sync.dma_start(out=dbg_g[:], in_=g[:, 0, :])
                            nc.sync.dma_start(
                                out=dbg_adt.rearrange(
                                    "p (s c) -> p s c", c=2),
                                in_=pe_sb[:])
                            nc.sync.dma_start(
                                out=dbg_ex[:],
                                in_=ex.rearrange("p s c -> p (s c)"))

                        pagg = aggp.tile([128, 258], f32, tag="agg")
                        for s in range(SW):
                            gsc = egs.tile([128, 258], bf16, tag="gsc")
                            nc.scalar.activation(gsc[:, 0:129],
                                                 g[:, s, 0:129],
                                                 AF.Copy,
                                                 scale=ex[:, s, 0:1])
                            if s % 4 == 3:
                                nc.scalar.activation(gsc[:, 129:258],
                                                     g[:, s, 129:258],
                                                     AF.Copy,
                                                     scale=ex[:, s, 1:2])
                            else:
                                nc.vector.tensor_scalar(
                                    out=gsc[:, 129:258],
                                    in0=g[:, s, 129:258],
                                    scalar1=ex[:, s, 1:2], scalar2=None,
                                    op0=OP.mult)
                            if DEBUG and conv == 0 and w == 0 and s == 0:
                                nc.sync.dma_start(out=dbg_oh[:],
                                                  in_=oh_all[:, 0, :])
                                nc.sync.dma_start(out=dbg_gsc[:], in_=gsc[:])
                            nc.tensor.matmul(pagg[:], lhsT=oh_all[:, s, :],
                                             rhs=gsc[:],
                                             start=(s == 0),
                                             stop=(s == SW - 1))

                        # h_new = 0.5*(msg0*rs0 + msg1*rs1) + bias
                        rs = em.tile([128, 2], f32, tag="rs")
                        nc.vector.tensor_scalar(
                            out=rs[:], in0=pagg[:].rearrange(
                                "p (a b) -> p a b", b=129)[:, :, 128:129],
                            scalar1=1e-16, scalar2=None, op0=OP.add)
                        nc.vector.reciprocal(rs[:], rs[:])
                        nc.vector.tensor_scalar(out=rs[:], in0=rs[:],
                                                scalar1=0.5, scalar2=None,
                                                op0=OP.mult)
                        t0 = em.tile([128, 128], f32, tag="t0")
                        nc.scalar.activation(t0[:], pagg[:, 0:128], AF.Copy,
                                             scale=rs[:, 0:1])
                        t1 = em.tile([128, 128], f32, tag="t1")
                        nc.scalar.activation(t1[:], pagg[:, 129:257], AF.Copy,
                                             scale=rs[:, 1:2])
                        hn = em.tile([128, 128], f32, tag="hn")
                        nc.vector.tensor_tensor(out=hn[:], in0=t0[:],
                                                in1=t1[:], op=OP.add)
                        nc.vector.tensor_tensor(out=hn[:], in0=hn[:],
                                                in1=bias_sb[:], op=OP.add)
                        if DEBUG and conv == 0 and w == 0:
                            pgc = em.tile([128, 258], f32, tag="pgc")
                            nc.vector.tensor_copy(pgc[:], pagg[:])
                            nc.sync.dma_start(out=dbg_pagg[:], in_=pgc[:])
                            nc.sync.dma_start(out=dbg_hn[:], in_=hn[:])
                        if DEBUG and conv == 1 and w == 0:
                            nc.sync.dma_start(out=dbg_hn1[:], in_=hn[:])
                        if DEBUG and conv == 2 and w == 0:
                            nc.sync.dma_start(out=dbg_hn2[:], in_=hn[:])
                        if conv < NUM_CONVS - 1:
                            pt = etp.tile([128, 128], f32, tag="pt")
                            nc.tensor.transpose(pt[:], hn[:], ident[:])
                            nc.vector.tensor_copy(
                                hT_sb[:, 128 * w:128 * w + nwn],
                                pt[:, 0:nwn])
                        else:
                            nc.sync.dma_start(
                                out=h_sh[128 * w:128 * w + nwn, :],
                                in_=hn[0:nwn, :])

                if conv == NUM_CONVS - 1:
                    nc.gpsimd.collective_compute(
                        "AllGather", mybir.AluOpType.bypass,
                        ins=[h_sh[:]], outs=[h3_full[:]],
                        replica_groups=[list(range(NCORES))])
                    if DEBUG:
                        nc.sync.dma_start(out=dbg_h3[:],
                                          in_=h3_full[0:128, :])

            # ---- set2set on this core's 16-graph slice ----
            with tc.tile_pool(name="s2s", bufs=1) as sp, \
                 tc.tile_pool(name="s2w", bufs=2) as swp, \
                 tc.tile_pool(name="s2p", bufs=2, space="PSUM") as s2p, \
                 tc.tile_pool(name="s2g", bufs=1, space="PSUM") as s2g:
                xloc = sp.tile([128, T, 128], f32)
                xidx_sb = sp.tile([128, T], mybir.dt.int32)
                nc.sync.dma_start(
                    out=xidx_sb[:],
                    in_=s2s_xidx.rearrange("t p o -> p (t o)"))
                for t in range(T):
                    nc.gpsimd.indirect_dma_start(
                        out=xloc[:, t, :], out_offset=None, in_=h3_full[:],
                        in_offset=bass.IndirectOffsetOnAxis(
                            ap=xidx_sb[:, t:t + 1], axis=0))
                bl = sp.tile([128, T], f32)
                nc.sync.dma_start(out=bl[:],
                                  in_=s2s_bloc.rearrange("t p o -> p (t o)"))
                brep_sb = sp.tile([16, T, 128], f32)
                nc.sync.dma_start(out=brep_sb[:],
                                  in_=s2s_brep.rearrange("t p d -> p t d"))
                oh = sp.tile([128, T, 16], f32)
                ohT = sp.tile([16, T, 128], f32)
                for t in range(T):
                    nc.vector.tensor_scalar(
                        out=oh[:, t, :], in0=iota16_row[:],
                        scalar1=bl[:, t:t + 1], scalar2=None, op0=OP.is_equal)
                    nc.vector.tensor_scalar(
                        out=ohT[:, t, :], in0=brep_sb[:, t, :],
                        scalar1=iota16_col[:], scalar2=None, op0=OP.is_equal)

                qT = sp.tile([128, 16], f32)
                nc.vector.memset(qT[:], 0.0)
                rT = sp.tile([128, 16], f32)
                nc.vector.memset(rT[:], 0.0)
                cst = sp.tile([16, 128], f32)
                nc.vector.memset(cst[:], 0.0)
                eloc = sp.tile([128, T], f32)

                for step in range(AGGR_STEPS):
                    pg = s2g.tile([16, 512], f32, tag="acc")
                    nc.tensor.matmul(pg[:], lhsT=qT[:], rhs=wia[:],
                                     start=True, stop=False)
                    nc.tensor.matmul(pg[:], lhsT=rT[:], rhs=wib[:],
                                     start=False, stop=False)
                    nc.tensor.matmul(pg[:], lhsT=qT[:], rhs=whh[:],
                                     start=False, stop=True)
                    gt = swp.tile([16, 512], f32, tag="gt")
                    nc.vector.tensor_tensor(out=gt[:], in0=pg[:], in1=bg_sb[:],
                                            op=OP.add)
                    sf = swp.tile([16, 128], f32, tag="sf")
                    nc.scalar.activation(sf[:], gt[:, 128:256], AF.Sigmoid)
                    si_ = swp.tile([16, 128], f32, tag="si")
                    nc.scalar.activation(si_[:], gt[:, 0:128], AF.Sigmoid)
                    tg = swp.tile([16, 128], f32, tag="tg")
                    nc.scalar.activation(tg[:], gt[:, 256:384], AF.Tanh)
                    so = swp.tile([16, 128], f32, tag="so")
                    nc.scalar.activation(so[:], gt[:, 384:512], AF.Sigmoid)
                    c2 = swp.tile([16, 128], f32, tag="c2")
                    nc.vector.tensor_tensor(out=c2[:], in0=sf[:], in1=cst[:],
                                            op=OP.mult)
                    it_ = swp.tile([16, 128], f32, tag="it")
                    nc.vector.tensor_tensor(out=it_[:], in0=si_[:], in1=tg[:],
                                            op=OP.mult)
                    nc.vector.tensor_tensor(out=c2[:], in0=c2[:], in1=it_[:],
                                            op=OP.add)
                    nc.vector.tensor_copy(cst[:], c2[:])
                    tc2 = swp.tile([16, 128], f32, tag="tc2")
                    nc.scalar.activation(tc2[:], c2[:], AF.Tanh)
                    qpad = swp.tile([128, 128], f32, tag="qpad")
                    nc.vector.memset(qpad[:], 0.0)
                    nc.vector.tensor_tensor(out=qpad[0:16, :], in0=so[:],
                                            in1=tc2[:], op=OP.mult)
                    ptq = s2p.tile([128, 128], f32, tag="tp")
                    nc.tensor.transpose(ptq[:], qpad[:], ident[:])
                    nc.vector.tensor_copy(qT[:], ptq[:, 0:16])

                    # e_n = x_n . q[batch_n]
                    for t in range(T):
                        pqx = s2p.tile([128, 128], f32, tag="tp")
                        nc.tensor.matmul(pqx[:], lhsT=ohT[:, t, :],
                                         rhs=qpad[0:16, :], start=True,
                                         stop=True)
                        xq = swp.tile([128, 128], f32, tag="xq")
                        nc.vector.scalar_tensor_tensor(
                            out=xq[:], in0=xloc[:, t, :], scalar=1.0,
                            in1=pqx[:], op0=OP.mult, op1=OP.mult,
                            accum_out=eloc[:, t:t + 1])
                    # global (per-core) max for stability
                    mx = swp.tile([128, 1], f32, tag="mx")
                    nc.vector.tensor_reduce(out=mx[:], in_=eloc[:],
                                            axis=mybir.AxisListType.X,
                                            op=OP.max)
                    mpad = swp.tile([128, 128], f32, tag="mpad")
                    nc.vector.memset(mpad[:], -1e30)
                    nc.vector.tensor_copy(mpad[:, 0:1], mx[:])
                    ptm = s2p.tile([128, 128], f32, tag="tp")
                    nc.tensor.transpose(ptm[:], mpad[:], ident[:])
                    msc = swp.tile([1, 1], f32, tag="msc")
                    nc.vector.tensor_reduce(out=msc[:], in_=ptm[0:1, :],
                                            axis=mybir.AxisListType.X,
                                            op=OP.max)
                    pnm = s2p.tile([128, 1], f32, tag="tp")
                    nc.tensor.matmul(pnm[:], lhsT=negones_row[:], rhs=msc[:],
                                     start=True, stop=True)
                    negm = swp.tile([128, 1], f32, tag="negm")
                    nc.vector.tensor_copy(negm[:], pnm[:])

                    pr = s2g.tile([16, 129], f32, tag="acc")
                    for t in range(T):
                        ev = swp.tile([128, 1], f32, tag="ev")
                        nc.scalar.activation(ev[:], eloc[:, t:t + 1], AF.Exp,
                                             bias=negm[:, 0:1])
                        msg = swp.tile([128, 129], f32, tag="msg")
                        nc.scalar.activation(msg[:, 0:128], xloc[:, t, :],
                                             AF.Copy, scale=ev[:, 0:1])
                        nc.vector.tensor_copy(msg[:, 128:129], ev[:])
                        nc.tensor.matmul(pr[:], lhsT=oh[:, t, :], rhs=msg[:],
                                         start=(t == 0), stop=(t == T - 1))
                    rsum = swp.tile([16, 1], f32, tag="rsum")
                    nc.vector.tensor_scalar(out=rsum[:], in0=pr[:, 128:129],
                                            scalar1=1e-16, scalar2=None,
                                            op0=OP.add)
                    nc.vector.reciprocal(rsum[:], rsum[:])
                    rpad = swp.tile([128, 128], f32, tag="rpad")
                    nc.vector.memset(rpad[:], 0.0)
                    nc.vector.tensor_scalar(out=rpad[0:16, :],
                                            in0=pr[:, 0:128],
                                            scalar1=rsum[:, 0:1],
                                            scalar2=None, op0=OP.mult)
                    ptr = s2p.tile([128, 128], f32, tag="tp")
                    nc.tensor.transpose(ptr[:], rpad[:], ident[:])
                    nc.vector.tensor_copy(rT[:], ptr[:, 0:16])

                # MLP head
                pm1 = s2g.tile([16, 128], f32, tag="acc")
                nc.tensor.matmul(pm1[:], lhsT=qT[:], rhs=w1a_sb[:],
                                 start=True, stop=False)
                nc.tensor.matmul(pm1[:], lhsT=rT[:], rhs=w1b_sb[:],
                                 start=False, stop=True)
                hidp = swp.tile([128, 128], f32, tag="hidp")
                nc.vector.memset(hidp[:], 0.0)
                nc.vector.tensor_tensor(out=hidp[0:16, :], in0=pm1[:],
                                        in1=b1_sb[:], op=OP.add)
                nc.scalar.activation(hidp[0:16, :], hidp[0:16, :], AF.Relu)
                pth = s2p.tile([128, 128], f32, tag="tp")
                nc.tensor.transpose(pth[:], hidp[:], ident[:])
                hT_m = swp.tile([128, 16], f32, tag="hTm")
                nc.vector.tensor_copy(hT_m[:], pth[:, 0:16])
                pm2 = s2g.tile([16, 128], f32, tag="acc")
                nc.tensor.matmul(pm2[:], lhsT=hT_m[:], rhs=w2_sb[:],
                                 start=True, stop=True)
                osb = swp.tile([16, 128], f32, tag="osb")
                nc.vector.tensor_tensor(out=osb[:], in0=pm2[:], in1=b2_sb[:],
                                        op=OP.add)
                nc.sync.dma_start(out=out[:], in_=osb[:])

    nc.compile()
    _split_waits(nc)
    return nc


# ---------------------------------------------------------------- entry
def kernel(x, edge_index, edge_attr, batch_index,
           gat_W, gat_att_src, gat_att_dst, gat_bias,
           lstm_Wih, lstm_Whh, lstm_bih, lstm_bhh,
           mlp_W1, mlp_b1, mlp_W2, mlp_b2, _trace=False):
    del edge_attr
    x = np.asarray(x, np.float32)
    edge_index = np.asarray(edge_index)
    batch_index = np.asarray(batch_index)

    cfg, per_core = _host_prep(x, edge_index, batch_index)
    W_eff, h0T = _prep_weights(x, gat_W, gat_att_src, gat_att_dst)

    Wih = np.asarray(lstm_Wih, np.float32)     # [512, 256]
    Whh = np.asarray(lstm_Whh, np.float32)     # [512, 128]
    WihT = Wih.T.copy()                        # [256, 512]
    bias_gates = (np.asarray(lstm_bih, np.float32)
                  + np.asarray(lstm_bhh, np.float32))
    common = dict(
        W_eff=W_eff,
        bias_rep=np.tile(np.asarray(gat_bias, np.float32)[None, :],
                         (128, 1)),
        WihT_a=WihT[0:128], WihT_b=WihT[128:256],
        WhhT=Whh.T.copy(),
        bg_rep=np.tile(bias_gates[None, :], (16, 1)),
        W1a=np.asarray(mlp_W1, np.float32)[0:128],
        W1b=np.asarray(mlp_W1, np.float32)[128:256],
        W2=np.asarray(mlp_W2, np.float32),
        b1_rep=np.tile(np.asarray(mlp_b1, np.float32)[None, :], (16, 1)),
        b2_rep=np.tile(np.asarray(mlp_b2, np.float32)[None, :], (16, 1)),
    )

    key = (cfg["SA"], cfg["SB"], cfg["T"])
    if _cached.get("key") != key:
        _cached["nc"] = _build(cfg)
        _cached["key"] = key
    nc = _cached["nc"]

    in_maps = []
    for c in range(NCORES):
        m = dict(common)
        m.update(per_core[c])
        m.pop("didx", None)
        m["h0T"] = h0T[c]
        m = {k: np.ascontiguousarray(v) for k, v in m.items()}
        in_maps.append(m)

    from concourse.bass_utils import run_bass_kernel_spmd
    res = run_bass_kernel_spmd(nc, in_maps, core_ids=list(range(NCORES)),
                               trace=_trace)
    outp = np.concatenate([res.results[c]["out"] for c in range(NCORES)],
                          axis=0)
    _cached["last_res"] = res
    if _trace:
        _cached["last_exec_ns"] = res.exec_time_ns
    return outp
